# revision 5
# baseline (speedup 1.0000x reference)
"""Trainium2 Bass kernel for ConcentrationLoss.

Math (per batch element b, fully independent across b):
    g      = grid[b] viewed as (2, 4096)            # channels x pixels
    coord1 = g @ aff[b]                             # (2, 4096), the heavy op
    view coord1 as (2, 64, 64); extract 8x8 windows stride 4 -> 15x15 windows
    loss contribution = sum over windows w of [ sum_{p in w} x_p^2 - (sum_{p in w} x_p)^2 / 64 ]
    final = sum_b contribution_b / (8 * 2 * 225 * 64)

Sharding: batch b -> core b (8 cores). Each core streams its 64MB aff slice
through the TensorEngine (memory-bound), reduces the windowed variance on
device to two small (30, 15) tensors (window sums S and window sums of
squares SSq), and the host combines the 8 partial results into the scalar.

Device layout notes:
  - Main matmul, orientation "A": out = lhsT.T @ rhs with lhsT = g^T chunks
    (128, 2) and rhs = aff row-band tiles (128, 512). coord1 accumulates in
    PSUM as (2, 4096) over the 32 contraction chunks.
  - coord1 (2, 4096) is bounced through a DRAM scratch to re-land in SBUF as
    X128 (128, 64) with partition = (channel, image row h), free = column w.
  - w-direction window sums: 8 strided vector adds -> Y (128, 15).
  - h-direction window sums + channel split: one matmul with a constant 0/1
    matrix RB (128, 30): S = RB.T @ Y -> (30, 15) = (channel*15 + i, j).
  - Same pipeline on X128^2 gives SSq.
"""

import numpy as np

B = 8
C = 2
H = W = 64
PIX = H * W  # 4096, contraction dim
WIN = 8
STRIDE = 4
OH = OW = 15
KC = PIX // 128  # 32 contraction chunks of 128
NT = PIX // 512  # 8 psum-bank-wide output chunks
KPT = 2          # contraction chunks per DMA tile (tile = 128 x KPT*4096 f32)
AFF_BUFS = 3

_CACHE = {}


def _split_multi_waits(nc, limit=1):
    """The walrus build in this toolchain rejects instructions carrying more
    than one sync wait (any template: CTRL, S3_LW, ...). Tile's scheduler
    freely emits multi-wait instructions. Post-process the scheduled BIR:
    hoist excess waits onto one-wait NoOps inserted immediately before the
    instruction on the same engine (sequencer waits are conjunctive and
    blocking, so semantics are identical)."""
    import concourse.mybir as mybir

    n_split = 0
    for f in nc.m.functions:
        for b in f.blocks:
            insts = b.instructions  # live view
            i = 0
            while i < len(insts):
                inst = insts[i]
                si = inst.sync_info
                if si is not None and len(si.on_wait) > limit:
                    waits = list(si.on_wait)
                    extra, keep = waits[:-limit], waits[-limit:]
                    for w in extra:
                        nop = mybir.InstNoOp(name=f"SWS-{n_split}")
                        n_split += 1
                        nop.engine = inst.engine
                        nop.sync_info = mybir.SyncInfo(on_wait=[w], on_update=[])
                        insts.insert(i, nop)
                        i += 1
                    inst.sync_info = mybir.SyncInfo(
                        on_wait=keep, on_update=si.on_update
                    )
                i += 1
    return n_split


def _build_nc():
    import concourse.bass as bass
    import concourse.mybir as mybir
    import concourse.tile as tile

    f32 = mybir.dt.float32
    nc = bass.Bass()
    aff = nc.dram_tensor("aff", [PIX, PIX], f32, kind="ExternalInput")
    gt = nc.dram_tensor("gt", [128, 2 * KC], f32, kind="ExternalInput")
    rb = nc.dram_tensor("rb", [128, 2 * OH], f32, kind="ExternalInput")
    out = nc.dram_tensor("out", [2 * OH, 2 * OW], f32, kind="ExternalOutput")
    scratch = nc.dram_tensor("scratch", [C, PIX], f32)

    with tile.TileContext(nc) as tc:
        with (
            tc.tile_pool(name="consts", bufs=1) as consts,
            tc.tile_pool(name="small", bufs=1) as small,
        ):
            gt_sb = consts.tile([128, 2 * KC], f32)
            nc.sync.dma_start(out=gt_sb, in_=gt[:, :])
            rb_sb = consts.tile([128, 2 * OH], f32)
            nc.sync.dma_start(out=rb_sb, in_=rb[:, :])
            c1_sb = small.tile([C, PIX], f32)

            ntiles = KC // KPT
            with (
                tc.tile_pool(name="affp", bufs=AFF_BUFS) as affp,
                tc.tile_pool(name="ps1", bufs=1, space="PSUM") as ps1,
            ):
                c1_ps = ps1.tile([C, PIX], f32)
                for t in range(ntiles):
                    at = affp.tile([128, KPT, PIX], f32)
                    src = aff[t * KPT * 128:(t + 1) * KPT * 128, :].rearrange(
                        "(j p) n -> p j n", p=128
                    )
                    nc.sync.dma_start(out=at, in_=src)
                    for j in range(KPT):
                        kc = t * KPT + j
                        for n in range(NT):
                            nc.tensor.matmul(
                                c1_ps[:, n * 512:(n + 1) * 512],
                                lhsT=gt_sb[:, 2 * kc:2 * kc + 2],
                                rhs=at[:, j, n * 512:(n + 1) * 512],
                                start=(kc == 0),
                                stop=(kc == KC - 1),
                            )
                nc.vector.tensor_copy(out=c1_sb[:, 0:2048], in_=c1_ps[:, 0:2048])
                nc.scalar.copy(out=c1_sb[:, 2048:4096], in_=c1_ps[:, 2048:4096])

            # reshape (2, 4096) -> (128, 64) via DRAM bounce
            nc.sync.dma_start(out=scratch[:, :], in_=c1_sb)
            x128 = small.tile([128, W], f32)
            nc.sync.dma_start(
                out=x128, in_=scratch[:, :].rearrange("c (h w) -> (c h) w", w=W)
            )

            sq = small.tile([128, W], f32)
            nc.scalar.square(out=sq, in_=x128)

            y = small.tile([128, OW], f32)
            ysq = small.tile([128, OW], f32)
            x3 = x128.rearrange("p (a f) -> p a f", f=STRIDE)
            s3 = sq.rearrange("p (a f) -> p a f", f=STRIDE)
            for dw in range(WIN):
                a, bb = divmod(dw, STRIDE)
                xs = x3[:, a:a + OW, bb]
                ss = s3[:, a:a + OW, bb]
                if dw == 0:
                    nc.vector.tensor_copy(out=y, in_=xs)
                    nc.vector.tensor_copy(out=ysq, in_=ss)
                else:
                    nc.vector.tensor_add(out=y, in0=y, in1=xs)
                    nc.vector.tensor_add(out=ysq, in0=ysq, in1=ss)

            with tc.tile_pool(name="ps2", bufs=1, space="PSUM") as ps2:
                s_ps = ps2.tile([2 * OH, OW], f32)
                ssq_ps = ps2.tile([2 * OH, OW], f32)
                nc.tensor.matmul(s_ps, lhsT=rb_sb, rhs=y, start=True, stop=True)
                nc.tensor.matmul(ssq_ps, lhsT=rb_sb, rhs=ysq, start=True, stop=True)
                out_sb = small.tile([2 * OH, 2 * OW], f32)
                nc.scalar.copy(out=out_sb[:, 0:OW], in_=s_ps)
                nc.scalar.copy(out=out_sb[:, OW:2 * OW], in_=ssq_ps)
            nc.sync.dma_start(out=out[:, :], in_=out_sb)
    _split_multi_waits(nc)
    return nc


def _rb_host():
    rb = np.zeros((128, 2 * OH), np.float32)
    for c in range(C):
        for i in range(OH):
            rb[c * H + STRIDE * i:c * H + STRIDE * i + WIN, c * OH + i] = 1.0
    return rb


def _gt_host(grid_b):
    # grid_b: (64, 64, 2). g[c, p] = grid_b.reshape(4096, 2)[p, c]
    # gt layout: gt[p, 2*kc + c] = g[c, 128*kc + p]
    gt = np.ascontiguousarray(grid_b, dtype=np.float32).reshape(PIX, C)
    return np.ascontiguousarray(
        gt.reshape(KC, 128, C).transpose(1, 0, 2).reshape(128, 2 * KC)
    )


def run_cores(aff, grid, trace=False):
    """Compile (cached) and run the per-core bass kernel on cores 0..7.

    Returns the BassKernelResults from run_bass_kernel_spmd."""
    from concourse.bass_utils import run_bass_kernel_spmd

    if "nc" not in _CACHE:
        _CACHE["nc"] = _build_nc()
    nc = _CACHE["nc"]

    rb = _rb_host()
    in_maps = []
    for b in range(B):
        in_maps.append(
            {
                "aff": np.ascontiguousarray(aff[b], dtype=np.float32),
                "gt": _gt_host(grid[b]),
                "rb": rb,
            }
        )
    return run_bass_kernel_spmd(nc, in_maps, core_ids=list(range(B)), trace=trace)


def kernel(aff, grid):
    aff = np.asarray(aff, dtype=np.float32)
    grid = np.asarray(grid, dtype=np.float32)
    res = run_cores(aff, grid)
    total = 0.0
    for b in range(B):
        o = res.results[b]["out"].astype(np.float64)
        s = o[:, 0:OW]
        ssq = o[:, OW:2 * OW]
        total += ssq.sum() - (s * s).sum() / (WIN * WIN)
    total /= B * C * OH * OW * WIN * WIN
    return np.asarray(total, dtype=np.float32)


# revision 7
# speedup vs baseline: 1.1318x; 1.1318x over previous
"""Trainium2 Bass kernel for ConcentrationLoss.

Math (per batch element b, fully independent across b):
    g      = grid[b] viewed as (2, 4096)            # channels x pixels
    coord1 = g @ aff[b]                             # (2, 4096), the heavy op
    view coord1 as (2, 64, 64); extract 8x8 windows stride 4 -> 15x15 windows
    loss contribution = sum over windows w of [ sum_{p in w} x_p^2 - (sum_{p in w} x_p)^2 / 64 ]
    final = sum_b contribution_b / (8 * 2 * 225 * 64)

Sharding: batch b -> core b (8 cores). Each core streams its 64MB aff slice
through the TensorEngine (memory-bound), reduces the windowed variance on
device to two small (30, 15) tensors (window sums S and window sums of
squares SSq), and the host combines the 8 partial results into the scalar.

Device layout notes:
  - Main matmul, orientation "A": out = lhsT.T @ rhs with lhsT = g^T chunks
    (128, 2) and rhs = aff row-band tiles (128, 512). coord1 accumulates in
    PSUM as (2, 4096) over the 32 contraction chunks.
  - coord1 (2, 4096) is bounced through a DRAM scratch to re-land in SBUF as
    X128 (128, 64) with partition = (channel, image row h), free = column w.
  - w-direction window sums: 8 strided vector adds -> Y (128, 15).
  - h-direction window sums + channel split: one matmul with a constant 0/1
    matrix RB (128, 30): S = RB.T @ Y -> (30, 15) = (channel*15 + i, j).
  - Same pipeline on X128^2 gives SSq.
"""

import numpy as np

B = 8
C = 2
H = W = 64
PIX = H * W  # 4096, contraction dim
WIN = 8
STRIDE = 4
OH = OW = 15
KC = PIX // 128  # 32 contraction chunks of 128
NT = PIX // 512  # 8 psum-bank-wide output chunks
KPT = 2          # contraction chunks per DMA tile (tile = 128 x KPT*4096 f32)
AFF_BUFS = 4
USE_F32R = True  # fp32r matmul: 1 cycle/col (vs fp32's 4) at ~2.8e-4 operand rounding

_CACHE = {}


def _split_multi_waits(nc, limit=1):
    """The walrus build in this toolchain rejects instructions carrying more
    than one sync wait (any template: CTRL, S3_LW, ...). Tile's scheduler
    freely emits multi-wait instructions. Post-process the scheduled BIR:
    hoist excess waits onto one-wait NoOps inserted immediately before the
    instruction on the same engine (sequencer waits are conjunctive and
    blocking, so semantics are identical)."""
    import concourse.mybir as mybir

    n_split = 0
    for f in nc.m.functions:
        for b in f.blocks:
            insts = b.instructions  # live view
            i = 0
            while i < len(insts):
                inst = insts[i]
                si = inst.sync_info
                if si is not None and len(si.on_wait) > limit:
                    waits = list(si.on_wait)
                    extra, keep = waits[:-limit], waits[-limit:]
                    for w in extra:
                        nop = mybir.InstNoOp(name=f"SWS-{n_split}")
                        n_split += 1
                        nop.engine = inst.engine
                        nop.sync_info = mybir.SyncInfo(on_wait=[w], on_update=[])
                        insts.insert(i, nop)
                        i += 1
                    inst.sync_info = mybir.SyncInfo(
                        on_wait=keep, on_update=si.on_update
                    )
                i += 1
    return n_split


def _build_nc():
    import concourse.bass as bass
    import concourse.mybir as mybir
    import concourse.tile as tile

    f32 = mybir.dt.float32
    fmm = mybir.dt.float32r if USE_F32R else f32
    nc = bass.Bass()
    aff = nc.dram_tensor("aff", [PIX, PIX], fmm, kind="ExternalInput")
    gt = nc.dram_tensor("gt", [128, 2 * KC], fmm, kind="ExternalInput")
    rb = nc.dram_tensor("rb", [128, 2 * OH], f32, kind="ExternalInput")
    out = nc.dram_tensor("out", [2 * OH, 2 * OW], f32, kind="ExternalOutput")
    scratch = nc.dram_tensor("scratch", [C, PIX], f32)

    with tile.TileContext(nc) as tc:
        with (
            tc.tile_pool(name="consts", bufs=1) as consts,
            tc.tile_pool(name="small", bufs=1) as small,
        ):
            # consts go through SWDGE (gpsimd) so they never queue behind the
            # big aff stream on the HWDGE rings
            gt_sb = consts.tile([128, 2 * KC], fmm)
            nc.gpsimd.dma_start(out=gt_sb, in_=gt[:, :])
            rb_sb = consts.tile([128, 2 * OH], f32)
            nc.gpsimd.dma_start(out=rb_sb, in_=rb[:, :])
            c1_sb = small.tile([C, PIX], f32)

            ntiles = KC // KPT
            with (
                tc.tile_pool(name="affp", bufs=AFF_BUFS) as affp,
                tc.tile_pool(name="ps1", bufs=1, space="PSUM") as ps1,
            ):
                c1_ps = ps1.tile([C, PIX], f32)
                for t in range(ntiles):
                    at = affp.tile([128, KPT, PIX], fmm)
                    src = aff[t * KPT * 128:(t + 1) * KPT * 128, :].rearrange(
                        "(j p) n -> p j n", p=128
                    )
                    nc.sync.dma_start(out=at, in_=src)
                    for j in range(KPT):
                        kc = t * KPT + j
                        for n in range(NT):
                            nc.tensor.matmul(
                                c1_ps[:, n * 512:(n + 1) * 512],
                                lhsT=gt_sb[:, 2 * kc:2 * kc + 2],
                                rhs=at[:, j, n * 512:(n + 1) * 512],
                                start=(kc == 0),
                                stop=(kc == KC - 1),
                            )
                # per-bank copies overlap with the tail matmuls
                for n in range(NT):
                    eng = nc.vector if n % 2 == 0 else nc.scalar
                    if n % 2 == 0:
                        eng.tensor_copy(
                            out=c1_sb[:, n * 512:(n + 1) * 512],
                            in_=c1_ps[:, n * 512:(n + 1) * 512],
                        )
                    else:
                        eng.copy(
                            out=c1_sb[:, n * 512:(n + 1) * 512],
                            in_=c1_ps[:, n * 512:(n + 1) * 512],
                        )

            # reshape (2, 4096) -> (128, 64) via DRAM bounce
            nc.sync.dma_start(out=scratch[:, :], in_=c1_sb)
            x128 = small.tile([128, W], f32)
            nc.sync.dma_start(
                out=x128, in_=scratch[:, :].rearrange("c (h w) -> (c h) w", w=W)
            )

            sq = small.tile([128, W], f32)
            nc.scalar.square(out=sq, in_=x128)

            y = small.tile([128, OW], f32)
            ysq = small.tile([128, OW], f32)
            x3 = x128.rearrange("p (a f) -> p a f", f=STRIDE)
            s3 = sq.rearrange("p (a f) -> p a f", f=STRIDE)
            for dw in range(WIN):
                a, bb = divmod(dw, STRIDE)
                xs = x3[:, a:a + OW, bb]
                ss = s3[:, a:a + OW, bb]
                if dw == 0:
                    nc.vector.tensor_copy(out=y, in_=xs)
                    nc.vector.tensor_copy(out=ysq, in_=ss)
                else:
                    nc.vector.tensor_add(out=y, in0=y, in1=xs)
                    nc.vector.tensor_add(out=ysq, in0=ysq, in1=ss)

            with tc.tile_pool(name="ps2", bufs=1, space="PSUM") as ps2:
                s_ps = ps2.tile([2 * OH, OW], f32)
                ssq_ps = ps2.tile([2 * OH, OW], f32)
                nc.tensor.matmul(s_ps, lhsT=rb_sb, rhs=y, start=True, stop=True)
                nc.tensor.matmul(ssq_ps, lhsT=rb_sb, rhs=ysq, start=True, stop=True)
                out_sb = small.tile([2 * OH, 2 * OW], f32)
                nc.scalar.copy(out=out_sb[:, 0:OW], in_=s_ps)
                nc.scalar.copy(out=out_sb[:, OW:2 * OW], in_=ssq_ps)
            nc.sync.dma_start(out=out[:, :], in_=out_sb)
    _split_multi_waits(nc)
    return nc


def _rb_host():
    rb = np.zeros((128, 2 * OH), np.float32)
    for c in range(C):
        for i in range(OH):
            rb[c * H + STRIDE * i:c * H + STRIDE * i + WIN, c * OH + i] = 1.0
    return rb


def _gt_host(grid_b):
    # grid_b: (64, 64, 2). g[c, p] = grid_b.reshape(4096, 2)[p, c]
    # gt layout: gt[p, 2*kc + c] = g[c, 128*kc + p]
    gt = np.ascontiguousarray(grid_b, dtype=np.float32).reshape(PIX, C)
    return np.ascontiguousarray(
        gt.reshape(KC, 128, C).transpose(1, 0, 2).reshape(128, 2 * KC)
    )


def run_cores(aff, grid, trace=False):
    """Compile (cached) and run the per-core bass kernel on cores 0..7.

    Returns the BassKernelResults from run_bass_kernel_spmd."""
    from concourse.bass_utils import run_bass_kernel_spmd

    if "nc" not in _CACHE:
        _CACHE["nc"] = _build_nc()
    nc = _CACHE["nc"]

    rb = _rb_host()
    in_maps = []
    for b in range(B):
        in_maps.append(
            {
                "aff": np.ascontiguousarray(aff[b], dtype=np.float32),
                "gt": _gt_host(grid[b]),
                "rb": rb,
            }
        )
    return run_bass_kernel_spmd(nc, in_maps, core_ids=list(range(B)), trace=trace)


def kernel(aff, grid):
    aff = np.asarray(aff, dtype=np.float32)
    grid = np.asarray(grid, dtype=np.float32)
    res = run_cores(aff, grid)
    total = 0.0
    for b in range(B):
        o = res.results[b]["out"].astype(np.float64)
        s = o[:, 0:OW]
        ssq = o[:, OW:2 * OW]
        total += ssq.sum() - (s * s).sum() / (WIN * WIN)
    total /= B * C * OH * OW * WIN * WIN
    return np.asarray(total, dtype=np.float32)


# revision 10
# speedup vs baseline: 1.1572x; 1.0224x over previous
"""Trainium2 Bass kernel for ConcentrationLoss.

Math (per batch element b, fully independent across b):
    g      = grid[b] viewed as (2, 4096)            # channels x pixels
    coord1 = g @ aff[b]                             # (2, 4096), the heavy op
    view coord1 as (2, 64, 64); extract 8x8 windows stride 4 -> 15x15 windows
    loss contribution = sum over windows w of [ sum_{p in w} x_p^2 - (sum_{p in w} x_p)^2 / 64 ]
    final = sum_b contribution_b / (8 * 2 * 225 * 64)

Sharding: batch b -> core b (8 cores). Each core streams its 64MB aff slice
through the TensorEngine (memory-bound), reduces the windowed variance on
device to per-channel partial sums, and the host combines the 8 partial
results into the scalar.

Device pipeline per core:
  - Main matmul: out = lhsT.T @ rhs with lhsT = g^T chunks (128, 2) and
    rhs = aff row-band tiles (128, 512), accumulated into PSUM (2, 4096)
    over the 32 contraction chunks. Operands are float32r: single-pass
    matmul at 1 col/cycle (fp32 would stream aff through the PE twice at
    half rate). fp32r rounds operands to ~12 mantissa bits; the end-to-end
    loss error stays ~1e-6 because the truncation noise averages out over
    the 230K-element mean.
  - As soon as PSUM bank n (512 pixels = 8 image rows) finishes
    accumulating, its post-processing overlaps the remaining stream:
    square (ACT), w-direction window sums of x and x^2 (8 strided adds
    each on DVE / GpSimd) into Y/Ysq (2, 64, 15).
  - After the last bank: h-direction window sums (8 strided adds) give
    S/SSq (2, 15, 15); then sum(SSq) and sum(S^2) reduce to a (2, 2)
    output. Host: loss_b = sum_c [ sumSSq_c - sumS2_c / 64 ].
"""

import numpy as np

B = 8
C = 2
H = W = 64
PIX = H * W  # 4096, contraction dim
WIN = 8
STRIDE = 4
OH = OW = 15
KC = PIX // 128  # 32 contraction chunks of 128
NT = PIX // 512  # 8 psum-bank-wide output chunks
ROWS_PER_BANK = 512 // W  # 8 image rows per psum bank
KPT = 2          # contraction chunks per DMA tile (tile = 128 x KPT*4096 f32)
AFF_BUFS = 4
USE_F32R = True  # fp32r matmul: 1 cycle/col (vs fp32's 4) at ~2.8e-4 operand rounding

_CACHE = {}


def _split_multi_waits(nc, limit=1):
    """The walrus build in this toolchain rejects instructions carrying more
    than one sync wait (any template: CTRL, S3_LW, ...). Tile's scheduler
    freely emits multi-wait instructions. Post-process the scheduled BIR:
    hoist excess waits onto one-wait NoOps inserted immediately before the
    instruction on the same engine (sequencer waits are conjunctive and
    blocking, so semantics are identical)."""
    import concourse.mybir as mybir

    n_split = 0
    for f in nc.m.functions:
        for b in f.blocks:
            insts = b.instructions  # live view
            i = 0
            while i < len(insts):
                inst = insts[i]
                si = inst.sync_info
                if si is not None and len(si.on_wait) > limit:
                    waits = list(si.on_wait)
                    extra, keep = waits[:-limit], waits[-limit:]
                    for w in extra:
                        nop = mybir.InstNoOp(name=f"SWS-{n_split}")
                        n_split += 1
                        nop.engine = inst.engine
                        nop.sync_info = mybir.SyncInfo(on_wait=[w], on_update=[])
                        insts.insert(i, nop)
                        i += 1
                    inst.sync_info = mybir.SyncInfo(
                        on_wait=keep, on_update=si.on_update
                    )
                i += 1
    return n_split


def _build_nc():
    import concourse.bass as bass
    import concourse.mybir as mybir
    import concourse.tile as tile

    f32 = mybir.dt.float32
    fmm = mybir.dt.float32r if USE_F32R else f32
    nc = bass.Bass()
    aff = nc.dram_tensor("aff", [PIX, PIX], fmm, kind="ExternalInput")
    gt = nc.dram_tensor("gt", [128, 2 * KC], fmm, kind="ExternalInput")
    out = nc.dram_tensor("out", [C, 2], f32, kind="ExternalOutput")

    with tile.TileContext(nc) as tc:
        with (
            tc.tile_pool(name="consts", bufs=1) as consts,
            tc.tile_pool(name="small", bufs=1) as small,
            tc.tile_pool(name="sqp", bufs=2) as sqp,
            tc.tile_pool(name="affp", bufs=AFF_BUFS) as affp,
            tc.tile_pool(name="ps1", bufs=1, space="PSUM") as ps1,
        ):
            # consts go through SWDGE (gpsimd) so they never queue behind the
            # big aff stream on the HWDGE rings
            gt_sb = consts.tile([128, 2 * KC], fmm)
            nc.gpsimd.dma_start(out=gt_sb, in_=gt[:, :])

            y_sb = small.tile([C, H, OW], f32)      # w-windowsums of x
            ysq_sb = small.tile([C, H, OW], f32)    # w-windowsums of x^2
            s_sb = small.tile([C, OH * OW], f32)    # full window sums
            ssq_sb = small.tile([C, OH * OW], f32)  # full window sums of x^2
            s2_sb = small.tile([C, OH * OW], f32)   # S^2
            out_sb = small.tile([C, 2], f32)

            c1_ps = ps1.tile([C, PIX], f32)
            ntiles = KC // KPT

            def bank_postprocess(n):
                """w-direction window sums for psum bank n; overlaps stream."""
                bank = c1_ps[:, n * 512:(n + 1) * 512]
                sq = sqp.tile([C, 512], f32, tag="sq")
                nc.scalar.square(out=sq, in_=bank)
                x4 = bank.rearrange("c (r q s) -> c r q s", r=ROWS_PER_BANK, s=STRIDE)
                q4 = sq.rearrange("c (r q s) -> c r q s", r=ROWS_PER_BANK, s=STRIDE)
                yd = y_sb[:, n * ROWS_PER_BANK:(n + 1) * ROWS_PER_BANK, :]
                qd = ysq_sb[:, n * ROWS_PER_BANK:(n + 1) * ROWS_PER_BANK, :]
                for dw in range(WIN):
                    a, bb = divmod(dw, STRIDE)
                    xs = x4[:, :, a:a + OW, bb]
                    qs = q4[:, :, a:a + OW, bb]
                    if dw == 0:
                        nc.vector.tensor_copy(out=yd, in_=xs)
                        nc.gpsimd.tensor_copy(out=qd, in_=qs)
                    else:
                        nc.vector.tensor_add(out=yd, in0=yd, in1=xs)
                        nc.gpsimd.tensor_add(out=qd, in0=qd, in1=qs)

            for t in range(ntiles):
                at = affp.tile([128, KPT, PIX], fmm)
                src = aff[t * KPT * 128:(t + 1) * KPT * 128, :].rearrange(
                    "(j p) n -> p j n", p=128
                )
                nc.sync.dma_start(out=at, in_=src)
                for j in range(KPT):
                    kc = t * KPT + j
                    for n in range(NT):
                        nc.tensor.matmul(
                            c1_ps[:, n * 512:(n + 1) * 512],
                            lhsT=gt_sb[:, 2 * kc:2 * kc + 2],
                            rhs=at[:, j, n * 512:(n + 1) * 512],
                            start=(kc == 0),
                            stop=(kc == KC - 1),
                        )
                        if kc == KC - 1:
                            bank_postprocess(n)

            # h-direction window sums: S[c, i, j] = sum_dh Y[c, 4i+dh, j]
            y4 = y_sb.rearrange("c (q r) j -> c q r j", r=STRIDE)
            q4 = ysq_sb.rearrange("c (q r) j -> c q r j", r=STRIDE)
            sv = s_sb.rearrange("c (i j) -> c i j", j=OW)
            qv = ssq_sb.rearrange("c (i j) -> c i j", j=OW)
            for dh in range(WIN):
                a, bb = divmod(dh, STRIDE)
                ys = y4[:, a:a + OH, bb, :]
                qs = q4[:, a:a + OH, bb, :]
                if dh == 0:
                    nc.vector.tensor_copy(out=sv, in_=ys)
                    nc.gpsimd.tensor_copy(out=qv, in_=qs)
                else:
                    nc.vector.tensor_add(out=sv, in0=sv, in1=ys)
                    nc.gpsimd.tensor_add(out=qv, in0=qv, in1=qs)

            nc.scalar.square(out=s2_sb, in_=s_sb)
            nc.vector.reduce_sum(out=out_sb[:, 0:1], in_=ssq_sb, axis=mybir.AxisListType.X)
            nc.vector.reduce_sum(out=out_sb[:, 1:2], in_=s2_sb, axis=mybir.AxisListType.X)
            nc.sync.dma_start(out=out[:, :], in_=out_sb)
    _split_multi_waits(nc)
    return nc


def _gt_host(grid_b):
    # grid_b: (64, 64, 2). g[c, p] = grid_b.reshape(4096, 2)[p, c]
    # gt layout: gt[p, 2*kc + c] = g[c, 128*kc + p]
    gt = np.ascontiguousarray(grid_b, dtype=np.float32).reshape(PIX, C)
    return np.ascontiguousarray(
        gt.reshape(KC, 128, C).transpose(1, 0, 2).reshape(128, 2 * KC)
    )


def run_cores(aff, grid, trace=False):
    """Compile (cached) and run the per-core bass kernel on cores 0..7.

    Returns the BassKernelResults from run_bass_kernel_spmd."""
    from concourse.bass_utils import run_bass_kernel_spmd

    if "nc" not in _CACHE:
        _CACHE["nc"] = _build_nc()
    nc = _CACHE["nc"]

    in_maps = []
    for b in range(B):
        in_maps.append(
            {
                "aff": np.ascontiguousarray(aff[b], dtype=np.float32),
                "gt": _gt_host(grid[b]),
            }
        )
    return run_bass_kernel_spmd(nc, in_maps, core_ids=list(range(B)), trace=trace)


def kernel(aff, grid):
    aff = np.asarray(aff, dtype=np.float32)
    grid = np.asarray(grid, dtype=np.float32)
    res = run_cores(aff, grid)
    total = 0.0
    for b in range(B):
        o = res.results[b]["out"].astype(np.float64)
        total += o[:, 0].sum() - o[:, 1].sum() / (WIN * WIN)
    total /= B * C * OH * OW * WIN * WIN
    return np.asarray(total, dtype=np.float32)
